# revision 11
# baseline (speedup 1.0000x reference)
"""GCN layer (x@Wn aggregated over edges + x@Ws + bias) on 8 Trainium2 cores.

Math: out[i] = sum_{(j->i)} w_ij * (x[j] @ W_nbrs) + x[i] @ W_self + bias
    = (sum_{(j->i)} w_ij * x[j]) @ W_nbrs + x[i] @ W_self + bias   (linearity)

Strategy (dst-sharded, one SPMD program on 8 cores, per-core data):
 - nodes split into 8 contiguous ranges of 12500; core c owns edges with
   dst in its range and produces out rows for its range.
 - x (bf16) replicated in HBM on every core as the gather source.
 - per core, edges grouped by (dst_tile, src_chunk); within each group
   the DISTINCT sources are gathered once (dedup) and the selection
   matrix S accumulates all of a source's (dst, w) contributions.
 - group row counts are padded only to the max over cores (shared SPMD
   program), NOT to 128 multiples: matmul blocks are sliding 128-row
   windows over each chunk's row list, so a block at a tile boundary is
   consumed by both tiles with S zeroing the foreign rows. This cuts
   gather descriptors ~16% vs 128-aligned groups.
 - gathers: segments of 2048 rows (16 blocks) per chunk, issued
   round-robin across SWDGE queues 0-3 so descriptor generation runs on
   all four Q7 core pairs concurrently (8ns/idx -> ~2.1ns/idx).
 - S matrices are built on the host (bf16, one [128 x 128] per matmul
   instance in consumption order) and streamed from HBM.
 - per dst tile: matmul(psumA += Xg_blk.T @ S_inst) accumulating
   aggT = [feat, slot] over the tile's block instances in PSUM.
 - project: psumB = aggT.T @ W_nbrs + xT_tile.T @ W_self (two matmuls),
   add bias via rank-1 matmul, DMA the [128 nodes, 128] f32 tile out.
"""
import sys

sys.path.insert(0, "/opt/trn_rl_repo")

import numpy as np
import ml_dtypes

import concourse.bacc as bacc
import concourse.mybir as mybir
from concourse.bass_utils import run_bass_kernel_spmd
from concourse.tile import TileContext

BF16 = mybir.dt.bfloat16
F32 = mybir.dt.float32
I16 = mybir.dt.int16
nbf = ml_dtypes.bfloat16

N = 100000
E = 1600000
D = 128
NC = 8
NPC = N // NC              # 12500 nodes per core
TPC = (NPC + 127) // 128   # 98 dst tiles per core
NPAD = TPC * 128           # 12544 padded nodes per core
CH = 4
CHROWS = 25000             # x rows per src chunk (< 2**15)
SEGBLK = 16                # blocks per gather segment (2048 rows)
SEGROWS = SEGBLK * 128
NQ = 4                     # SWDGE queues for gather desc-gen
SSEG = 32                  # S-matrix instances per streamed S segment


def _preprocess(edge_src, edge_dst, edge_weight):
    src = np.asarray(edge_src, dtype=np.int64)
    dst = np.asarray(edge_dst, dtype=np.int64)
    wgt = np.asarray(edge_weight, dtype=np.float32)

    core = dst // NPC
    tile = (dst % NPC) // 128
    chunk = src // CHROWS

    # ---- shared structure: per-group max unique-src count over cores ----
    ucnt = np.zeros((NC, TPC * CH), dtype=np.int64)
    per_core_sorted = []
    for c in range(NC):
        sel = core == c
        t_c = tile[sel]
        k_c = chunk[sel]
        s_c = src[sel] % CHROWS
        d_c = (dst[sel] % NPC) % 128
        w_c = wgt[sel]
        o = np.lexsort((s_c, t_c, k_c))
        t_c, k_c, s_c, d_c, w_c = t_c[o], k_c[o], s_c[o], d_c[o], w_c[o]
        per_core_sorted.append((t_c, k_c, s_c, d_c, w_c))
        key = (k_c * TPC + t_c) * 32768 + s_c
        newu = np.ones(len(key), dtype=bool)
        newu[1:] = key[1:] != key[:-1]
        np.add.at(ucnt[c], (k_c * TPC + t_c)[newu], 1)
    M = ucnt.max(axis=0).reshape(CH, TPC).T.copy()  # [TPC, CH] rows per group

    # chunk-relative row offsets r0[t,k]; chunk padded to SEGROWS multiple.
    # Segment plan per chunk: first and last segments are 1024 rows (8
    # blocks) so the serialized first dispatches are short and the final
    # segment unlocks fewer trailing tiles; middle segments are 2048 rows.
    r0 = np.zeros((TPC, CH), dtype=np.int64)
    chunk_rows = np.zeros(CH, dtype=np.int64)      # padded rows per chunk
    chunk_rowbase = np.zeros(CH, dtype=np.int64)   # global slot base
    seg_plan = []                                  # per chunk: [(start_blk, nblk)]
    base = 0
    for k in range(CH):
        off = 0
        for t in range(TPC):
            r0[t, k] = off
            off += int(M[t, k])
        nfull = -(-off // SEGROWS)                 # 2048-row units
        rows = nfull * SEGROWS
        plan = [(0, 8)]
        b = 8
        while b + 16 <= rows // 128 - 8:
            plan.append((b, 16))
            b += 16
        while b < rows // 128:
            plan.append((b, 8))
            b += 8
        seg_plan.append(plan)
        chunk_rows[k] = rows
        chunk_rowbase[k] = base
        base += rows
    TOTROWS = int(base)
    chunk_nseg = np.array([len(p) for p in seg_plan], dtype=np.int64)
    NSEG = int(chunk_nseg.sum())

    # matmul instances: per (t, k) the sliding blocks overlapping the group
    jlo = np.zeros((TPC, CH), dtype=np.int64)
    ninst = np.zeros((TPC, CH), dtype=np.int64)
    inst_base = np.zeros((TPC, CH), dtype=np.int64)
    ib = 0
    for t in range(TPC):
        for k in range(CH):
            m = int(M[t, k])
            if m:
                lo = int(r0[t, k]) // 128
                hi = (int(r0[t, k]) + m - 1) // 128
                jlo[t, k] = lo
                ninst[t, k] = hi - lo + 1
                inst_base[t, k] = ib
                ib += hi - lo + 1
    NINST = int(ib)

    # ---- per-core data ----
    per_core = []
    for c in range(NC):
        t_c, k_c, s_c, d_c, w_c = per_core_sorted[c]
        key = (k_c * TPC + t_c) * 32768 + s_c
        newu = np.ones(len(key), dtype=bool)
        newu[1:] = key[1:] != key[:-1]
        # rank of the unique src within its group
        u_idx = np.cumsum(newu) - 1                  # unique id (global asc)
        grp = k_c * TPC + t_c
        gnew = np.ones(len(grp), dtype=bool)
        gnew[1:] = grp[1:] != grp[:-1]
        # rank within group = u_idx - (unique id at the group's first edge)
        first_u_per_edge = u_idx[gnew][np.cumsum(gnew) - 1]
        rank = u_idx - first_u_per_edge
        # chunk-relative row of each edge's source
        row = r0[t_c, k_c] + rank
        j = row // 128
        inst = inst_base[t_c, k_c] + (j - jlo[t_c, k_c])

        # S: [128 e, NINST, 128 slot] accumulated in f32
        S_arr = np.zeros((128, NINST, 128), dtype=np.float32)
        np.add.at(S_arr, (row % 128, inst, d_c), w_c)

        # idx per unique row (global slot space)
        gslot = chunk_rowbase[k_c] + row
        idx16 = np.zeros(TOTROWS, dtype=np.int16)
        idx16[gslot[newu]] = s_c[newu]
        idx_w = np.tile(idx16.reshape(-1, 16).T, (8, 1)).copy()  # [128, TOTROWS//16]

        per_core.append((idx_w, S_arr.astype(nbf).reshape(128, NINST * 128)))

    meta = dict(
        M=M, r0=r0, jlo=jlo, ninst=ninst, inst_base=inst_base,
        NINST=NINST, TOTROWS=TOTROWS, NSEG=NSEG,
        chunk_rows=chunk_rows, chunk_nseg=chunk_nseg, chunk_rowbase=chunk_rowbase,
        seg_plan=seg_plan,
    )
    return meta, per_core


def _build_program(meta):
    M = meta["M"]
    jlo = meta["jlo"]
    ninst = meta["ninst"]
    inst_base = meta["inst_base"]
    NINST = meta["NINST"]
    TOTROWS = meta["TOTROWS"]
    chunk_nseg = meta["chunk_nseg"]
    chunk_rows = meta["chunk_rows"]
    seg_plan = meta["seg_plan"]
    # block -> segment index maps
    seg_of_blk = []
    for k in range(CH):
        m = np.zeros(int(chunk_rows[k]) // 128, dtype=np.int64)
        for si, (b0, nb) in enumerate(seg_plan[k]):
            m[b0 : b0 + nb] = si
        seg_of_blk.append(m)

    nc = bacc.Bacc(num_swdge_queues=NQ)
    x_bf = nc.declare_dram_parameter("x_bf", [N, D], BF16, isOutput=False)
    idx_d = nc.declare_dram_parameter("idx", [128, TOTROWS // 16], I16, isOutput=False)
    s_d = nc.declare_dram_parameter("smat", [128, NINST * 128], BF16, isOutput=False)
    wn_d = nc.declare_dram_parameter("wn", [128, 128], BF16, isOutput=False)
    ws_d = nc.declare_dram_parameter("ws", [128, 128], BF16, isOutput=False)
    xt_d = nc.declare_dram_parameter("xt", [128, NPAD], BF16, isOutput=False)
    bias_d = nc.declare_dram_parameter("bias_bc", [128, 128], F32, isOutput=False)
    out_d = nc.declare_dram_parameter("out", [NPAD, 128], F32, isOutput=True)

    # idx columns per chunk (wrapped layout: slot s -> column s//16)
    chunk_colbase = [int(meta["chunk_rowbase"][k]) // 16 for k in range(CH)]
    chunk_cols = [int(chunk_rows[k]) // 16 for k in range(CH)]

    with TileContext(nc) as tc:
        with (
            tc.tile_pool(name="const", bufs=1) as cpool,
            tc.tile_pool(name="gather", bufs=4) as gpool,
            tc.tile_pool(name="smats", bufs=5) as spool,
            tc.tile_pool(name="work", bufs=4) as wpool,
            tc.tile_pool(name="outp", bufs=3) as opool,
            tc.tile_pool(name="psA", bufs=2, space="PSUM") as pApool,
            tc.tile_pool(name="psB", bufs=2, space="PSUM") as pBpool,
        ):
            idx_ts = []
            for k in range(CH):
                it = cpool.tile([128, chunk_cols[k]], I16, name=f"idx{k}")
                nc.sync.dma_start(
                    out=it[:],
                    in_=idx_d[:, chunk_colbase[k] : chunk_colbase[k] + chunk_cols[k]],
                )
                idx_ts.append(it)
            wn_t = cpool.tile([128, 128], BF16)
            nc.sync.dma_start(out=wn_t[:], in_=wn_d[:])
            ws_t = cpool.tile([128, 128], BF16)
            nc.sync.dma_start(out=ws_t[:], in_=ws_d[:])
            xt_t = cpool.tile([128, NPAD], BF16)
            nc.sync.dma_start(out=xt_t[:], in_=xt_d[:])
            bias_t = cpool.tile([128, 128], F32)
            nc.sync.dma_start(out=bias_t[:], in_=bias_d[:])
            bias_bf = cpool.tile([1, 128], BF16)
            nc.vector.tensor_copy(out=bias_bf[:], in_=bias_t[0:1, :])
            ones_t = cpool.tile([1, 128], BF16)
            nc.vector.memset(ones_t[:], 1.0)

            seg_tiles = {}   # (k, seg) -> gather tile
            s_tiles = {}     # si -> S tile
            qn = [0]

            def issue_gather(k, seg):
                b0, nb = seg_plan[k][seg]
                nidx = nb * 128
                gt = gpool.tile([128, SEGROWS], BF16, tag=f"g{k}")
                nc.gpsimd.dma_gather(
                    out_ap=gt[:, 0 : nb * 128].rearrange("p (b e) -> p b e", e=128),
                    in_ap=x_bf[k * CHROWS : min((k + 1) * CHROWS, N), :],
                    idxs_ap=idx_ts[k][:, b0 * 8 : (b0 + nb) * 8],
                    num_idxs=nidx,
                    num_idxs_reg=nidx,
                    elem_size=128,
                    single_packet=False,
                    queue_num=(qn[0] + 1) % NQ,
                )
                qn[0] += 1
                seg_tiles[(k, seg)] = gt

            maxseg = int(chunk_nseg.max())
            order = [
                (k, s)
                for s in range(maxseg)
                for k in range(CH)
                if s < int(chunk_nseg[k])
            ]
            issued = 0

            def ensure_issued_through(k, seg):
                nonlocal issued
                while (k, seg) not in seg_tiles and issued < len(order):
                    issue_gather(*order[issued])
                    issued += 1

            n_sseg = -(-NINST // SSEG)
            s_issued = [0]

            def ensure_s_issued(inst):
                si = inst // SSEG
                while s_issued[0] <= si:
                    i0 = s_issued[0] * SSEG
                    w = min(SSEG, NINST - i0)
                    st = spool.tile([128, SSEG * 128], BF16, tag="s")
                    nc.sync.dma_start(
                        out=st[:, 0 : w * 128],
                        in_=s_d[:, i0 * 128 : (i0 + w) * 128],
                    )
                    s_tiles[s_issued[0]] = st
                    s_issued[0] += 1

            for t in range(TPC):
                # per-tile matmul instances (k, seg, local_blk, inst_id)
                tile_parts = []
                for k in range(CH):
                    for i in range(int(ninst[t, k])):
                        j = int(jlo[t, k]) + i
                        si = int(seg_of_blk[k][j])
                        lb = j - seg_plan[k][si][0]
                        tile_parts.append(
                            (k, si, lb, int(inst_base[t, k]) + i)
                        )

                for k, seg, _, _ in tile_parts:
                    ensure_issued_through(k, seg)
                for _, _, _, inst in tile_parts:
                    ensure_s_issued(inst)

                psumB = pBpool.tile([128, 128], F32, space="PSUM", tag="psB")
                if tile_parts:
                    psumA = pApool.tile([128, 128], F32, space="PSUM", tag="psA")
                    for jj, (k, seg, lb, inst) in enumerate(tile_parts):
                        gt = seg_tiles[(k, seg)]
                        st = s_tiles[inst // SSEG]
                        il = inst % SSEG
                        nc.tensor.matmul(
                            out=psumA[:],
                            lhsT=gt[:, lb * 128 : (lb + 1) * 128],
                            rhs=st[:, il * 128 : (il + 1) * 128],
                            start=(jj == 0),
                            stop=(jj == len(tile_parts) - 1),
                        )
                    aggT = wpool.tile([128, 128], BF16, tag="aggT")
                    nc.scalar.copy(out=aggT[:], in_=psumA[:])
                    nc.tensor.matmul(
                        out=psumB[:], lhsT=aggT[:], rhs=wn_t[:],
                        start=True, stop=False,
                    )
                    nc.tensor.matmul(
                        out=psumB[:],
                        lhsT=xt_t[:, t * 128 : (t + 1) * 128],
                        rhs=ws_t[:],
                        start=False, stop=False,
                    )
                    nc.tensor.matmul(
                        out=psumB[:], lhsT=ones_t[:], rhs=bias_bf[:],
                        start=False, stop=True,
                    )
                else:
                    nc.tensor.matmul(
                        out=psumB[:],
                        lhsT=xt_t[:, t * 128 : (t + 1) * 128],
                        rhs=ws_t[:],
                        start=True, stop=False,
                    )
                    nc.tensor.matmul(
                        out=psumB[:], lhsT=ones_t[:], rhs=bias_bf[:],
                        start=False, stop=True,
                    )
                out_t = opool.tile([128, 128], F32, tag="out")
                nc.scalar.copy(out=out_t[:], in_=psumB[:])
                nc.sync.dma_start(
                    out=out_d[t * 128 : (t + 1) * 128, :], in_=out_t[:]
                )

    nc.compile()
    return nc


def kernel(x, edge_src, edge_dst, edge_weight, W_nbrs, W_self, bias, _trace=False,
           _tmpdir=None):
    x = np.asarray(x, dtype=np.float32)
    meta, per_core = _preprocess(edge_src, edge_dst, edge_weight)
    nc = _build_program(meta)

    x_bf = x.astype(nbf)
    wn = np.asarray(W_nbrs, dtype=np.float32).astype(nbf)
    ws = np.asarray(W_self, dtype=np.float32).astype(nbf)
    bias_bc = np.broadcast_to(np.asarray(bias, dtype=np.float32), (128, 128)).copy()

    in_maps = []
    for c in range(NC):
        idx_w, s_arr = per_core[c]
        xt = np.zeros((128, NPAD), dtype=np.float32)
        xt[:, :NPC] = x[c * NPC : (c + 1) * NPC].T
        in_maps.append(
            dict(
                x_bf=x_bf,
                idx=idx_w,
                smat=s_arr,
                wn=wn,
                ws=ws,
                xt=xt.astype(nbf),
                bias_bc=bias_bc,
            )
        )

    res = run_bass_kernel_spmd(
        nc, in_maps, list(range(NC)), trace=_trace, tmpdir=_tmpdir
    )
    out = np.empty((N, D), dtype=np.float32)
    for c in range(NC):
        out[c * NPC : (c + 1) * NPC] = res.results[c]["out"][:NPC]
    if _trace:
        kernel._last_result = res
    return out
